# revision 8
# baseline (speedup 1.0000x reference)
import sys
from contextlib import ExitStack

import numpy as np

sys.path.insert(0, "/opt/trn_rl_repo")

import concourse.bass as bass
import concourse.tile as tile
from concourse import bacc, mybir
from concourse.bass_utils import run_bass_kernel_spmd

B, H, W, CH = 4, 80, 80, 256
NCLS, DIM = 22, 256
ROWS = 40            # rows per core
NPIX = ROWS * W      # 3200 output pixels per core
NT = (ROWS + 2) * W + 2   # 3362 strip positions (1 halo row each side + 1 elem pad)
NTILE = NPIX // 128  # 25 output tiles of 128 pixels
F32 = mybir.dt.float32
F32R = mybir.dt.float32r


def _build_nc():
    nc = bacc.Bacc("TRN2", target_bir_lowering=False, debug=False,
                   enable_asserts=True, num_devices=8)
    xt_d = nc.dram_tensor("xt", [128, 2 * NT], F32R, kind="ExternalInput").ap()
    wt_d = nc.dram_tensor("wt", [128, 18 * DIM], F32R, kind="ExternalInput").ap()
    seg_d = nc.dram_tensor("segs", [128, NTILE, 9, NCLS], F32,
                           kind="ExternalInput").ap()
    out_d = nc.dram_tensor("out", [NPIX, DIM], F32, kind="ExternalOutput").ap()

    with tile.TileContext(nc) as tc, ExitStack() as ctx:
        xp = ctx.enter_context(tc.tile_pool(name="xp", bufs=1))
        wp = ctx.enter_context(tc.tile_pool(name="wp", bufs=1))
        sp = ctx.enter_context(tc.tile_pool(name="sp", bufs=1))
        selp_pool = ctx.enter_context(tc.tile_pool(name="selp_pool", bufs=3))
        accp = ctx.enter_context(tc.tile_pool(name="accp", bufs=3))
        zp = ctx.enter_context(tc.tile_pool(name="zp", bufs=6, space="PSUM"))

        xt = xp.tile([128, 2 * NT], F32R)
        wt = wp.tile([128, 18 * DIM], F32R)
        segs = sp.tile([128, NTILE, 9, NCLS], F32)

        # weights first (every matmul needs them), 18 pieces for fine deps
        for i in range(18):
            nc.sync.dma_start(wt[:, i * DIM:(i + 1) * DIM],
                              wt_d[:, i * DIM:(i + 1) * DIM])
        # x in 4 chunks per half so early tiles can start sooner
        bnds = [0, 850, 1700, 2550, NT]
        for ci in range(4):
            for h in range(2):
                a, b = h * NT + bnds[ci], h * NT + bnds[ci + 1]
                nc.sync.dma_start(xt[:, a:b], xt_d[:, a:b])
        for j in range(NTILE):
            nc.sync.dma_start(segs[:, j], seg_d[:, j])

        for j in range(NTILE):
            m = segs[:, j, 4]                       # [128, 22] center
            smax = selp_pool.tile([128, 1], F32)
            nc.vector.tensor_reduce(smax[:], m, axis=mybir.AxisListType.X,
                                    op=mybir.AluOpType.max)
            sel = selp_pool.tile([128, 9], F32)
            trash = selp_pool.tile([128, NCLS], F32)
            for k in range(9):
                # (m == smax) * seg_k ; accum_out -> sel[:, k]
                nc.vector.scalar_tensor_tensor(
                    trash[:], m, smax[:], segs[:, j, k],
                    op0=mybir.AluOpType.is_equal, op1=mybir.AluOpType.mult,
                    accum_out=sel[:, k:k + 1])
            trash9 = selp_pool.tile([128, 9], F32)
            cnt = selp_pool.tile([128, 1], F32)
            nc.vector.tensor_scalar(trash9[:], sel[:], 0.0, None,
                                    op0=mybir.AluOpType.not_equal,
                                    op1=mybir.AluOpType.add,
                                    accum_out=cnt[:])
            cntc = selp_pool.tile([128, 1], F32)
            nc.vector.tensor_scalar(cntc[:], cnt[:], 1.0, None,
                                    op0=mybir.AluOpType.max)
            rec = selp_pool.tile([128, 1], F32)
            nc.vector.reciprocal(rec[:], cntc[:])
            selp = selp_pool.tile([128, 9], F32)
            nc.vector.tensor_scalar(selp[:], sel[:], rec[:], 9.0,
                                    op0=mybir.AluOpType.mult,
                                    op1=mybir.AluOpType.mult)

            acc = accp.tile([128, DIM], F32)
            for k in range(9):
                base = j * 128 + 80 * (k // 3) + (k % 3)
                z = zp.tile([128, DIM], F32)
                nc.tensor.matmul(z[:], xt[:, base:base + 128],
                                 wt[:, k * DIM:(k + 1) * DIM],
                                 start=True, stop=False)
                nc.tensor.matmul(z[:], xt[:, NT + base:NT + base + 128],
                                 wt[:, (9 + k) * DIM:(10 + k) * DIM],
                                 start=False, stop=True)
                if k == 0:
                    nc.vector.tensor_scalar(acc[:], z[:], selp[:, 0:1], None,
                                            op0=mybir.AluOpType.mult)
                else:
                    nc.vector.scalar_tensor_tensor(
                        acc[:], z[:], selp[:, k:k + 1], acc[:],
                        op0=mybir.AluOpType.mult, op1=mybir.AluOpType.add)
            nc.sync.dma_start(out_d[j * 128:(j + 1) * 128, :], acc[:])
    nc.compile()
    return nc


_NC_CACHE = None


def _get_nc():
    global _NC_CACHE
    if _NC_CACHE is None:
        _NC_CACHE = _build_nc()
    return _NC_CACHE


def _prep_core(x, seg_mask, core):
    b, r0 = core // 2, 40 * (core % 2)
    xp = np.pad(x[b], ((1, 1), (0, 0), (0, 0)))        # [82,80,256]
    strip = xp[r0:r0 + 42].reshape(42 * W, CH)
    sp = np.zeros((NT, CH), np.float32)
    sp[1:1 + 42 * W] = strip
    spT = sp.T
    xt = np.ascontiguousarray(np.concatenate([spT[:128], spT[128:]], axis=1))

    pads = np.pad(seg_mask[b], ((1, 1), (1, 1), (0, 0)))  # [82,82,22]
    vs = []
    for k in range(9):
        di, dj = k // 3 - 1, k % 3 - 1
        vs.append(pads[r0 + 1 + di:r0 + 41 + di, 1 + dj:81 + dj, :]
                  .reshape(NPIX, NCLS))
    segs = (np.stack(vs, axis=1).reshape(NTILE, 128, 9, NCLS)
            .transpose(1, 0, 2, 3))
    return xt, np.ascontiguousarray(segs)


def kernel(x, seg_mask, conv_w):
    x = np.asarray(x, np.float32)
    seg_mask = np.asarray(seg_mask, np.float32)
    conv_w = np.asarray(conv_w, np.float32)

    w9 = conv_w.reshape(CH, 9, DIM)
    wt = np.ascontiguousarray(np.concatenate(
        [w9[:128].reshape(128, 9 * DIM), w9[128:].reshape(128, 9 * DIM)],
        axis=1))

    in_maps = []
    for core in range(8):
        xt, segs = _prep_core(x, seg_mask, core)
        in_maps.append({"xt": xt, "wt": wt, "segs": segs})

    nc = _get_nc()
    res = run_bass_kernel_spmd(nc, in_maps, core_ids=list(range(8)))

    out = np.empty((B, H, W, DIM), np.float32)
    for core in range(8):
        b, r0 = core // 2, 40 * (core % 2)
        out[b, r0:r0 + 40] = res.results[core]["out"].reshape(ROWS, W, DIM)
    return out
